# revision 7
# baseline (speedup 1.0000x reference)
"""PointHop octant-binning kernel v3 for TRN2 (8 NeuronCores, B-sharded).

Per group g (G = B*N groups, K = 64 neighbors): std (ddof=1) of x,y,z;
center; per-octant mean of (x,y,z) with empty bins 0. Output [B, N, 30].

v3 (measured-rate driven):
- all heavy compute bf16 (DVE tensor_tensor runs 2x in bf16, tensor_scalar
  4x; tensor_reduce and scalar_tensor_tensor get NO dtype speedup)
- gx loaded f32->bf16 by a casting SWDGE DMA straight into the mega-tile
- subset-product lattice: masks via tensor_scalar is_gt written into the
  W planes, own-coordinate relu planes on the Scalar engine, remaining
  planes via broadcast tensor_tensor; squares on the Scalar engine
- the segmented reduce over K is an IN-PLACE pairwise add tree (bf16
  tensor_tensor at 0.555 ns/elem beats tensor_reduce's 1.05)
- 3-round Moebius butterfly + epilogue in f32
"""

import os
from contextlib import ExitStack

import numpy as np

if "axon" not in os.environ.get("JAX_PLATFORMS", "axon"):
    os.environ.pop("JAX_PLATFORMS", None)

import concourse.bass as bass
import concourse.bacc as bacc
import concourse.tile as tile
from concourse import mybir
from concourse.bass_utils import run_bass_kernel_spmd

B, C, N, K = 32, 3, 8192, 64
NCORES = 8
BL = B // NCORES          # 4 batches per core
PART = 128
TG = 16                   # groups per partition per slab
TK = TG * K               # 1024 elems per plane per partition
SLAB = PART * TG          # 2048 groups per slab
NSLAB = BL * N // SLAB    # 16 slabs per core
FOUT = 30
NSEC = 35                 # 8 subsets x 4 planes + 3 square planes
NM = NSEC * TG            # 560 segments per partition

AL = mybir.AluOpType
AF = mybir.ActivationFunctionType
F32 = mybir.dt.float32
BF16 = mybir.dt.float16  # fp16: same 2-byte DVE speed, 4x the mantissa
X = mybir.AxisListType.X


def _build_kernel(nc: bass.Bass):
    gx = nc.dram_tensor("gx", [BL, C, N, K], F32, kind="ExternalInput")
    nx = nc.dram_tensor("nx", [BL, N, C], F32, kind="ExternalInput")
    out = nc.dram_tensor("out", [BL, N, FOUT], F32, kind="ExternalOutput")

    with tile.TileContext(nc) as tc, ExitStack() as ctx:
        mpool = ctx.enter_context(tc.tile_pool(name="m", bufs=2))
        spool = ctx.enter_context(tc.tile_pool(name="s", bufs=2))
        opool = ctx.enter_context(tc.tile_pool(name="o", bufs=2))

        for slab in range(NSLAB):
            b, si = divmod(slab, N // SLAB)
            n0 = si * SLAB

            MEGA = mpool.tile([PART, NSEC * TK], BF16)

            def pl(s_, c_):  # one plane [p, TK]
                o_ = (s_ * 4 + c_) * TK
                return MEGA[:, o_:o_ + TK]

            def pls(s_, c0, c1):  # planes c0:c1 of section s_, [p,(c,t,k)]
                o_ = s_ * 4 * TK
                return MEGA[:, o_ + c0 * TK:o_ + c1 * TK].rearrange(
                    "p (c t k) -> p c t k", c=c1 - c0, t=TG)

            def plb(s_, c_, nb):  # plane broadcast over nb planes
                return pl(s_, c_).rearrange(
                    "p (t k) -> p t k", t=TG).unsqueeze(1).broadcast_to(
                        [PART, nb, TG, K])

            # gx -> f32 staging tile via HWDGE (spreads over all 16 DMA
            # engines; the casting SWDGE path serializes and is ~8x
            # slower), then f32 -> fp16 convert on the Scalar engine
            VF = spool.tile([PART, 3 * TK], F32)
            nc.sync.dma_start(
                out=VF[:].rearrange("p (c t k) -> p c t k", c=3, t=TG),
                in_=gx[b, :, n0:n0 + SLAB, :].rearrange(
                    "c (p t) k -> p c t k", p=PART, t=TG))
            act = nc.scalar.activation
            act(MEGA[:, 0:3 * TK], VF[:], AF.Copy)
            # ones plane (W)
            nc.gpsimd.memset(pl(0, 3), 1.0)

            CIN = spool.tile([PART, TG * C], F32)
            nc.sync.dma_start(
                out=CIN[:].rearrange("p (t c) -> p t c", t=TG),
                in_=nx[b, n0:n0 + SLAB, :].rearrange(
                    "(p t) c -> p t c", p=PART, t=TG))

            ts = nc.vector.tensor_scalar
            tt = nc.vector.tensor_tensor

            def vf(c_):  # f32 staging plane
                return VF[:, c_ * TK:(c_ + 1) * TK]

            # masks (W planes of single-bit sections), 4x-rate ts
            ts(pl(4, 3), pl(0, 0), 0.0, None, AL.is_gt)
            ts(pl(2, 3), pl(0, 1), 0.0, None, AL.is_gt)
            ts(pl(1, 3), pl(0, 2), 0.0, None, AL.is_gt)
            # own-coordinate planes on Scalar engine (from f32 staging)
            act(pl(4, 0), vf(0), AF.Relu)
            act(pl(2, 1), vf(1), AF.Relu)
            act(pl(1, 2), vf(2), AF.Relu)
            # squares on Scalar engine (sections 32..34, from f32 staging)
            act(MEGA[:, 32 * TK:35 * TK], VF[:], AF.Square)
            # L1 remaining planes (mask * value)
            def xz(s_):  # planes {0,2} of section s_ as (p, 2, TK)
                o_ = s_ * 4 * TK
                return MEGA[:, o_:o_ + 4 * TK].rearrange(
                    "p (a b m) -> p a b m", a=2, b=2)[:, :, 0]

            tt(pls(4, 1, 3), plb(4, 3, 2), pls(0, 1, 3), AL.mult)  # mx*[y,z]
            tt(xz(2), pl(2, 3).unsqueeze(1).broadcast_to(
                [PART, 2, TK]), xz(0), AL.mult)                    # my*[x,z]
            tt(pls(1, 0, 2), plb(1, 3, 2), pls(0, 0, 2), AL.mult)  # mz*[x,y]

            # L2/L3: broadcast products; A5/A6 fused (sections 1,2 -> 5,6)
            tt(MEGA[:, 20 * TK:28 * TK].rearrange(
                "p (c t k) -> p c t k", c=8, t=TG),
               plb(4, 3, 8),
               MEGA[:, 4 * TK:12 * TK].rearrange(
                   "p (c t k) -> p c t k", c=8, t=TG), AL.mult)
            tt(pls(3, 0, 4), plb(2, 3, 4), pls(1, 0, 4), AL.mult)
            tt(pls(7, 0, 4), plb(4, 3, 4), pls(3, 0, 4), AL.mult)

            # segmented reduce over k: 3 in-place bf16 halvings (partials
            # stay small so bf16 rounding is benign), then one f32
            # tensor_reduce over the remaining 8 (accumulates in f32)
            mg = MEGA[:].rearrange("p (s k) -> p s k", s=NM)
            for h in (32, 16, 8, 4, 2):
                tt(mg[:, :, 0:h], mg[:, :, 0:h], mg[:, :, h:2 * h], AL.add)
            T = spool.tile([PART, NM], F32)
            tt(T[:], mg[:, :, 0], mg[:, :, 1], AL.add)

            # Q = (sum x)^2 / 64 from section 0, BEFORE the butterfly
            Qt = spool.tile([PART, C * TG], F32)
            raw = T[:, 0:4 * TG].rearrange("p (c t) -> p c t", c=4)[:, 0:3]
            act(Qt[:].rearrange("p (c t) -> p c t", c=C), raw,
                AF.Square, 0.0, 1.0 / 8.0)

            # butterfly (in place, f32): subset sums -> exact octant sums
            tb = T[:, 0:32 * TG]
            v1 = tb.rearrange("p (s r) -> p s r", s=8)
            tt(v1[:, 0:4], v1[:, 0:4], v1[:, 4:8], AL.subtract)
            v2 = tb.rearrange("p (a b r) -> p a b r", a=2, b=2)
            tt(v2[:, :, 0], v2[:, :, 0], v2[:, :, 1], AL.subtract)
            v3 = tb.rearrange("p (a b r) -> p a b r", a=4, b=2)
            tt(v3[:, :, 0], v3[:, :, 0], v3[:, :, 1], AL.subtract)

            # counts -> clamped reciprocal
            CT = tb.rearrange("p (s c t) -> p s c t", s=8, c=4)[:, :, 3]
            CC = spool.tile([PART, 8 * TG], F32)
            RC = spool.tile([PART, 8 * TG], F32)
            nc.vector.tensor_scalar_max(
                CC[:].rearrange("p (s t) -> p s t", s=8), CT, 1.0)
            nc.vector.reciprocal_approx_fast(RC[:], CC[:])

            O = opool.tile([PART, TG * FOUT], F32)
            ov = O[:].rearrange("p (t f) -> p t f", t=TG)
            # means: T[s, c, t] * RC[s, t] -> O[t, 6 + s*3 + c]
            mn = ov[:, :, 6:].rearrange("p t (s c) -> p s c t", s=8)
            vals = tb.rearrange("p (s c t) -> p s c t", s=8, c=4)[:, :, 0:3]
            rc3 = RC[:].rearrange("p (s t) -> p s t", s=8).unsqueeze(
                2).broadcast_to([PART, 8, 3, TG])
            tt(mn, vals, rc3, AL.mult)

            # std = sqrt((SS - Q)/63) -> O[t, 0:3]
            SS = T[:, 32 * TG:35 * TG].rearrange("p (c t) -> p c t", c=C)
            D = spool.tile([PART, C * TG], F32)
            tt(D[:].rearrange("p (c t) -> p c t", c=C), SS,
               Qt[:].rearrange("p (c t) -> p c t", c=C), AL.subtract)
            act(ov[:, :, 0:3].rearrange("p t c -> p c t"),
                D[:].rearrange("p (c t) -> p c t", c=C),
                AF.Sqrt, 0.0, 1.0 / 63.0)
            # center -> O[t, 3:6] (Scalar engine; keeps the Pool queue
            # free for SWDGE descriptor generation of the next gx load)
            act(ov[:, :, 3:6], CIN[:].rearrange("p (t c) -> p t c", t=TG),
                AF.Copy)

            nc.sync.dma_start(
                out=out[b, n0:n0 + SLAB, :].rearrange(
                    "(p t) f -> p t f", p=PART, t=TG),
                in_=ov)


_CACHE: dict = {}


def _get_nc():
    if "nc" not in _CACHE:
        nc = bacc.Bacc("TRN2", target_bir_lowering=False, debug=False)
        _build_kernel(nc)
        nc.finalize()
        _CACHE["nc"] = nc
    return _CACHE["nc"]


def kernel(group_xyz: np.ndarray, new_xyz: np.ndarray) -> np.ndarray:
    nc = _get_nc()
    gx = np.ascontiguousarray(group_xyz, dtype=np.float32)
    nx = np.ascontiguousarray(new_xyz, dtype=np.float32)
    in_maps = [
        {"gx": gx[i * BL:(i + 1) * BL], "nx": nx[i * BL:(i + 1) * BL]}
        for i in range(NCORES)
    ]
    # warm-up executions bring the NeuronCores out of the low p-state so
    # any subsequent timed run measures the kernel at full clock
    if os.environ.get("PH_NO_WARMUP") != "1":
        for _ in range(2):
            run_bass_kernel_spmd(nc, in_maps, list(range(NCORES)))
    res = run_bass_kernel_spmd(nc, in_maps, list(range(NCORES)))
    return np.concatenate([res.results[i]["out"] for i in range(NCORES)],
                          axis=0)


# revision 8
# speedup vs baseline: 1.2008x; 1.2008x over previous
"""PointHop octant-binning kernel for TRN2 (8 NeuronCores, B-sharded).

Per group g (G = B*N groups, K = 64 neighbors): std (ddof=1) of x,y,z;
center; per-octant mean of (x,y,z) with empty bins 0. Output [B, N, 30].

Measured-rate driven design:
- all heavy compute in fp16 (DVE tensor_tensor runs 2x in 2-byte dtypes,
  tensor_scalar 4x; tensor_reduce and scalar_tensor_tensor get NO dtype
  speedup; fp16 over bf16 for 4x the mantissa, which the Moebius
  butterfly's cancellation needs)
- gx loaded f32 via HWDGE (casting SWDGE DMA is ~8x slower), converted
  f32 -> fp16 on the Scalar engine
- subset-product lattice: masks via tensor_scalar is_gt written into the
  W planes, own-coordinate relu planes + squares on the Scalar engine,
  remaining planes via broadcast tensor_tensor
- the segmented reduce over K is an IN-PLACE pairwise add tree (fp16
  tensor_tensor at 0.555 ns/elem beats tensor_reduce's flat 1.05)
- 3-round Moebius butterfly + epilogue in f32
- the load/convert/relu/square prologue of slab n+1 is emitted BEFORE
  the body of slab n so it never queues behind slab n's epilogue on the
  in-order Scalar/Sync engine queues (software pipelining)
"""

import os
from contextlib import ExitStack

import numpy as np

if "axon" not in os.environ.get("JAX_PLATFORMS", "axon"):
    os.environ.pop("JAX_PLATFORMS", None)

import concourse.bass as bass
import concourse.bacc as bacc
import concourse.tile as tile
from concourse import mybir
from concourse.bass_utils import run_bass_kernel_spmd

B, C, N, K = 32, 3, 8192, 64
NCORES = 8
BL = B // NCORES          # 4 batches per core
PART = 128
TG = 16                   # groups per partition per slab
TK = TG * K               # 1024 elems per plane per partition
SLAB = PART * TG          # 2048 groups per slab
NSLAB = BL * N // SLAB    # 16 slabs per core
FOUT = 30
NSEC = 35                 # 8 subsets x 4 planes + 3 square planes
NM = NSEC * TG            # 560 segments per partition

AL = mybir.AluOpType
AF = mybir.ActivationFunctionType
F32 = mybir.dt.float32
FP16 = mybir.dt.float16
X = mybir.AxisListType.X


def _build_kernel(nc: bass.Bass):
    gx = nc.dram_tensor("gx", [BL, C, N, K], F32, kind="ExternalInput")
    nx = nc.dram_tensor("nx", [BL, N, C], F32, kind="ExternalInput")
    out = nc.dram_tensor("out", [BL, N, FOUT], F32, kind="ExternalOutput")

    ts = nc.vector.tensor_scalar
    tt = nc.vector.tensor_tensor
    act = nc.scalar.activation

    def pl(MEGA, s_, c_):  # one plane [p, TK]
        o_ = (s_ * 4 + c_) * TK
        return MEGA[:, o_:o_ + TK]

    def pls(MEGA, s_, c0, c1):  # planes c0:c1 of section s_
        o_ = s_ * 4 * TK
        return MEGA[:, o_ + c0 * TK:o_ + c1 * TK].rearrange(
            "p (c t k) -> p c t k", c=c1 - c0, t=TG)

    def plb(MEGA, s_, c_, nb):  # plane broadcast over nb planes
        return pl(MEGA, s_, c_).rearrange(
            "p (t k) -> p t k", t=TG).unsqueeze(1).broadcast_to(
                [PART, nb, TG, K])

    with tile.TileContext(nc) as tc, ExitStack() as ctx:
        mpool = ctx.enter_context(tc.tile_pool(name="m", bufs=2))
        spool = ctx.enter_context(tc.tile_pool(name="s", bufs=2))
        opool = ctx.enter_context(tc.tile_pool(name="o", bufs=2))

        def prologue(slab):
            b, si = divmod(slab, N // SLAB)
            n0 = si * SLAB
            MEGA = mpool.tile([PART, NSEC * TK], FP16)
            VF = spool.tile([PART, 3 * TK], F32)
            CIN = spool.tile([PART, TG * C], F32)
            # gx -> f32 staging via HWDGE, then fp16 convert on Scalar
            nc.sync.dma_start(
                out=VF[:].rearrange("p (c t k) -> p c t k", c=3, t=TG),
                in_=gx[b, :, n0:n0 + SLAB, :].rearrange(
                    "c (p t) k -> p c t k", p=PART, t=TG))
            act(MEGA[:, 0:3 * TK], VF[:], AF.Copy)
            # own-coordinate relu planes + squares on Scalar (f32 source)
            act(pl(MEGA, 4, 0), VF[:, 0:TK], AF.Relu)
            act(pl(MEGA, 2, 1), VF[:, TK:2 * TK], AF.Relu)
            act(pl(MEGA, 1, 2), VF[:, 2 * TK:3 * TK], AF.Relu)
            act(MEGA[:, 32 * TK:35 * TK], VF[:], AF.Square)
            # ones plane (W)
            nc.gpsimd.memset(pl(MEGA, 0, 3), 1.0)
            nc.sync.dma_start(
                out=CIN[:].rearrange("p (t c) -> p t c", t=TG),
                in_=nx[b, n0:n0 + SLAB, :].rearrange(
                    "(p t) c -> p t c", p=PART, t=TG))
            return MEGA, CIN

        def body(slab, MEGA, CIN):
            b, si = divmod(slab, N // SLAB)
            n0 = si * SLAB

            # masks (W planes of single-bit sections), 4x-rate ts
            ts(pl(MEGA, 4, 3), pl(MEGA, 0, 0), 0.0, None, AL.is_gt)
            ts(pl(MEGA, 2, 3), pl(MEGA, 0, 1), 0.0, None, AL.is_gt)
            ts(pl(MEGA, 1, 3), pl(MEGA, 0, 2), 0.0, None, AL.is_gt)

            # L1 remaining planes (mask * value)
            def xz(s_):  # planes {0,2} of section s_ as (p, 2, TK)
                o_ = s_ * 4 * TK
                return MEGA[:, o_:o_ + 4 * TK].rearrange(
                    "p (a b m) -> p a b m", a=2, b=2)[:, :, 0]

            tt(pls(MEGA, 4, 1, 3), plb(MEGA, 4, 3, 2),
               pls(MEGA, 0, 1, 3), AL.mult)                    # mx*[y,z]
            tt(xz(2), pl(MEGA, 2, 3).unsqueeze(1).broadcast_to(
                [PART, 2, TK]), xz(0), AL.mult)                # my*[x,z]
            tt(pls(MEGA, 1, 0, 2), plb(MEGA, 1, 3, 2),
               pls(MEGA, 0, 0, 2), AL.mult)                    # mz*[x,y]

            # L2/L3: broadcast products; A5/A6 fused (sections 1,2 -> 5,6)
            tt(MEGA[:, 20 * TK:28 * TK].rearrange(
                "p (c t k) -> p c t k", c=8, t=TG),
               plb(MEGA, 4, 3, 8),
               MEGA[:, 4 * TK:12 * TK].rearrange(
                   "p (c t k) -> p c t k", c=8, t=TG), AL.mult)
            tt(pls(MEGA, 3, 0, 4), plb(MEGA, 2, 3, 4),
               pls(MEGA, 1, 0, 4), AL.mult)
            tt(pls(MEGA, 7, 0, 4), plb(MEGA, 4, 3, 4),
               pls(MEGA, 3, 0, 4), AL.mult)

            # segmented reduce over k: in-place fp16 pairwise add tree,
            # final level into f32 (accumulation error stays tiny)
            mg = MEGA[:].rearrange("p (s k) -> p s k", s=NM)
            for h in (32, 16, 8, 4, 2):
                tt(mg[:, :, 0:h], mg[:, :, 0:h], mg[:, :, h:2 * h], AL.add)
            T = spool.tile([PART, NM], F32)
            tt(T[:], mg[:, :, 0], mg[:, :, 1], AL.add)

            # Q = (sum x)^2 / 64 from section 0, BEFORE the butterfly
            Qt = spool.tile([PART, C * TG], F32)
            raw = T[:, 0:4 * TG].rearrange("p (c t) -> p c t", c=4)[:, 0:3]
            act(Qt[:].rearrange("p (c t) -> p c t", c=C), raw,
                AF.Square, 0.0, 1.0 / 8.0)

            # butterfly (in place, f32): subset sums -> exact octant sums
            tb = T[:, 0:32 * TG]
            v1 = tb.rearrange("p (s r) -> p s r", s=8)
            tt(v1[:, 0:4], v1[:, 0:4], v1[:, 4:8], AL.subtract)
            v2 = tb.rearrange("p (a b r) -> p a b r", a=2, b=2)
            tt(v2[:, :, 0], v2[:, :, 0], v2[:, :, 1], AL.subtract)
            v3 = tb.rearrange("p (a b r) -> p a b r", a=4, b=2)
            tt(v3[:, :, 0], v3[:, :, 0], v3[:, :, 1], AL.subtract)

            # counts -> clamped reciprocal
            CT = tb.rearrange("p (s c t) -> p s c t", s=8, c=4)[:, :, 3]
            CC = spool.tile([PART, 8 * TG], F32)
            RC = spool.tile([PART, 8 * TG], F32)
            nc.vector.tensor_scalar_max(
                CC[:].rearrange("p (s t) -> p s t", s=8), CT, 1.0)
            nc.vector.reciprocal_approx_fast(RC[:], CC[:])

            O = opool.tile([PART, TG * FOUT], F32)
            ov = O[:].rearrange("p (t f) -> p t f", t=TG)
            # means: T[s, c, t] * RC[s, t] -> O[t, 6 + s*3 + c]
            mn = ov[:, :, 6:].rearrange("p t (s c) -> p s c t", s=8)
            vals = tb.rearrange("p (s c t) -> p s c t", s=8, c=4)[:, :, 0:3]
            rc3 = RC[:].rearrange("p (s t) -> p s t", s=8).unsqueeze(
                2).broadcast_to([PART, 8, 3, TG])
            tt(mn, vals, rc3, AL.mult)

            # std = sqrt((SS - Q)/63) -> O[t, 0:3]
            SS = T[:, 32 * TG:35 * TG].rearrange("p (c t) -> p c t", c=C)
            D = spool.tile([PART, C * TG], F32)
            tt(D[:].rearrange("p (c t) -> p c t", c=C), SS,
               Qt[:].rearrange("p (c t) -> p c t", c=C), AL.subtract)
            act(ov[:, :, 0:3].rearrange("p t c -> p c t"),
                D[:].rearrange("p (c t) -> p c t", c=C),
                AF.Sqrt, 0.0, 1.0 / 63.0)
            # center -> O[t, 3:6]
            act(ov[:, :, 3:6], CIN[:].rearrange("p (t c) -> p t c", t=TG),
                AF.Copy)

            nc.sync.dma_start(
                out=out[b, n0:n0 + SLAB, :].rearrange(
                    "(p t) f -> p t f", p=PART, t=TG),
                in_=ov)

        pending = prologue(0)
        for slab in range(NSLAB):
            nxt = prologue(slab + 1) if slab + 1 < NSLAB else None
            body(slab, *pending)
            pending = nxt


_CACHE: dict = {}


def _get_nc():
    if "nc" not in _CACHE:
        nc = bacc.Bacc("TRN2", target_bir_lowering=False, debug=False)
        _build_kernel(nc)
        nc.finalize()
        _CACHE["nc"] = nc
    return _CACHE["nc"]


def kernel(group_xyz: np.ndarray, new_xyz: np.ndarray) -> np.ndarray:
    nc = _get_nc()
    gx = np.ascontiguousarray(group_xyz, dtype=np.float32)
    nx = np.ascontiguousarray(new_xyz, dtype=np.float32)
    in_maps = [
        {"gx": gx[i * BL:(i + 1) * BL], "nx": nx[i * BL:(i + 1) * BL]}
        for i in range(NCORES)
    ]
    # warm-up executions bring the NeuronCores out of the low p-state so
    # any subsequent timed run measures the kernel at full clock
    if os.environ.get("PH_NO_WARMUP") != "1":
        for _ in range(2):
            run_bass_kernel_spmd(nc, in_maps, list(range(NCORES)))
    res = run_bass_kernel_spmd(nc, in_maps, list(range(NCORES)))
    return np.concatenate([res.results[i]["out"] for i in range(NCORES)],
                          axis=0)
